# revision 1
# baseline (speedup 1.0000x reference)
"""CVRP loss kernel for 8 Trainium2 NeuronCores.

Strategy (per spec sharding hint): shard the 6.4M edges across 8 cores.
Each core computes local in/out degree-prob bins over all 100K nodes via
one-hot matmuls accumulating in PSUM (node n = 128*hi + lo; lo selects the
PSUM partition through the stationary operand, hi selects the PSUM column
through the moving operand), plus streaming partial sums for the focal loss.
The [128, 782] bin grids and scalar partials are AllReduce'd across the 8
cores, then every core redundantly assembles the final scalar loss.

Self-contained: shapes hardcoded for the nn_CVRPLoss problem
(6.4M edges, 100K nodes).
"""
import numpy as np

import concourse.bass as bass
import concourse.mybir as mybir
from concourse.bass_utils import run_bass_kernel_spmd

P = 128                 # partitions
NF = 782                # node hi range: 128*782 = 100096 >= 100000
N_NODES = 100000
N_EDGES = 6400000
NCORES = 8
EPC = N_EDGES // NCORES      # 800000 edges per core
NCOLS_FULL = 6272            # padded cols per core: 6272*128 = 802816
U = 128                      # superchunk width (cols per load batch)
R = 8                        # build-tile ring depth
PAD_LOGIT = -60.0            # sigmoid(-60) == 0 -> padding edges contribute 0

F32 = mybir.dt.float32
F16 = mybir.dt.float16
BF16 = mybir.dt.bfloat16
I32 = mybir.dt.int32
Alu = mybir.AluOpType
Act = mybir.ActivationFunctionType


def build_nc(ncols=NCOLS_FULL, n_edges_real=N_EDGES, repeat=1):
    assert ncols % U == 0
    ns = ncols // U          # superchunks
    nchunk = ncols           # one chunk per column
    nc = bass.Bass()

    ep_ext = nc.declare_dram_parameter("ep", [P, ncols], F32, isOutput=False)
    ye_ext = nc.declare_dram_parameter("ye", [P, ncols], F32, isOutput=False)
    dlo_ext = nc.declare_dram_parameter("dlo", [P, ncols], F32, isOutput=False)
    dhi_ext = nc.declare_dram_parameter("dhi", [P, ncols], F32, isOutput=False)
    slo_ext = nc.declare_dram_parameter("slo", [P, ncols], F32, isOutput=False)
    shi_ext = nc.declare_dram_parameter("shi", [P, ncols], F32, isOutput=False)
    np_ext = nc.declare_dram_parameter("npred", [P, NF], F32, isOutput=False)
    yn_ext = nc.declare_dram_parameter("ynode", [P, NF], F32, isOutput=False)
    dem_ext = nc.declare_dram_parameter("dem", [P, NF], F32, isOutput=False)
    cap_ext = nc.declare_dram_parameter("cap", [1, 1], F32, isOutput=False)
    out_ext = nc.declare_dram_parameter("out", [1, 1], F32, isOutput=True)

    SW = 2 * NF + 16         # stats buffer width (bins + packed scalars)
    cc_in = nc.dram_tensor("cc_in", [P, SW], F32)
    cc_out = nc.dram_tensor("cc_out", [P, SW], F32)

    from contextlib import ExitStack
    es = ExitStack()
    mk = lambda name, shape, dt: es.enter_context(nc.sbuf_tensor(name, shape, dt))
    mkp = lambda name, shape, dt: es.enter_context(nc.psum_tensor(name, shape, dt))
    sem = lambda name: es.enter_context(nc.semaphore(name))
    # input double buffers
    b_ep = mk("b_ep", [P, 2 * U], F32); b_ye = mk("b_ye", [P, 2 * U], F32)
    b_dlo = mk("b_dlo", [P, 2 * U], F32); b_dhi = mk("b_dhi", [P, 2 * U], F32)
    b_slo = mk("b_slo", [P, 2 * U], F32); b_shi = mk("b_shi", [P, 2 * U], F32)
    # derived per-superchunk streams (double buffered)
    b_p = mk("b_p", [P, 2 * U], F32)      # sigmoid(ep)       (DVE)
    b_t = mk("b_t", [P, 2 * U], F32)      # exp(-|ep|)        (ACT)
    b_t1p = mk("b_t1p", [P, 2 * U], F32)  # 1 + t             (POOL)
    b_l = mk("b_l", [P, 2 * U], F32)      # ln(1+t)           (ACT)
    # engine-local scratch
    vwA = mk("vwA", [P, U], F32)          # ACT |x|
    vw1 = mk("vw1", [P, U], F32); vw2 = mk("vw2", [P, U], F32)
    vw3 = mk("vw3", [P, U], F32)          # DVE probs scratch
    w1 = mk("w1", [P, U], F32); w2 = mk("w2", [P, U], F32)
    w3 = mk("w3", [P, U], F32)            # POOL focal scratch
    facc = mk("facc", [P, U], F32)
    # one-hot build rings
    lhsT_d = mk("lhsT_d", [P, R * P], BF16); lhsT_s = mk("lhsT_s", [P, R * P], BF16)
    hi_d = mk("hi_d", [P, R * NF], BF16); hi_s = mk("hi_s", [P, R * NF], BF16)
    # constants
    iota128 = mk("iota128", [P, P], F16); iota782 = mk("iota782", [P, NF], F16)
    ones = mk("ones", [P, 1], F32)
    neg1 = mk("neg1", [P, 1], F32)
    # node tiles
    npred_t = mk("npred_t", [P, NF], F32); ynode_t = mk("ynode_t", [P, NF], F32)
    dem_t = mk("dem_t", [P, NF], F32)
    nf_w1 = mk("nf_w1", [P, NF], F32); nf_w2 = mk("nf_w2", [P, NF], F32)
    nf_trash = mk("nf_trash", [P, NF], BF16)
    # stats / final
    stats = mk("stats", [P, SW], F32); packed = mk("packed", [P, 8], F32)
    r8 = mk("r8", [1, 8], F32)
    capsb = mk("capsb", [1, 1], F32); sc = mk("sc", [1, 16], F32)
    i32t = mk("i32t", [1, 1], I32)
    outsb = mk("outsb", [1, 1], F32)
    # PSUM
    ps_in = mkp("ps_in", [P, 1024], F32); ps_out = mkp("ps_out", [P, 1024], F32)
    ps_fin = mkp("ps_fin", [1, 8], F32)
    # semaphores
    dma_semA = sem("dma_semA")  # SYNC loads, even superchunks: 96 each
    dma_semB = sem("dma_semB")  # SYNC loads, odd superchunks: 96 each
    nod_sem = sem("nod_sem")    # node/cap loads: 64 total
    actE_sem = sem("actE_sem")  # ACT exp done: s+1
    actL_sem = sem("actL_sem")  # ACT ln done: s+1
    aux_sem = sem("aux_sem")    # POOL 1+t done: s+1
    dvp_sem = sem("dvp_sem")    # DVE probs done: s+1
    pool_sem = sem("pool_sem")  # POOL focal done: s+1
    set_sem = sem("set_sem")    # POOL setup (iotas/facc): 1
    vset_sem = sem("vset_sem")  # DVE setup (ones/neg1): 1
    bld_sem = sem("bld_sem")    # DVE chunk builds: g+1
    pe_sem = sem("pe_sem")      # PE chunk matmuls: g+1
    cc_sem = sem("cc_sem")
    fin_sem = sem("fin_sem")
    odma_sem = sem("odma_sem")

    with es, nc.Block() as block:
        def ring(t, g, w):
            return t[:, (g % R) * w:(g % R + 1) * w]

        # ---------------- SYNC: all input DMA ----------------
        @block.sync
        def _(sync):
            sync.dma_start(out=npred_t[:, :], in_=np_ext[:, :]).then_inc(nod_sem, 16)
            sync.dma_start(out=ynode_t[:, :], in_=yn_ext[:, :]).then_inc(nod_sem, 16)
            sync.dma_start(out=dem_t[:, :], in_=dem_ext[:, :]).then_inc(nod_sem, 16)
            sync.dma_start(out=capsb[:, :], in_=cap_ext[:, :]).then_inc(nod_sem, 16)
            for s in range(ns * repeat):
                if s >= 2:
                    # in-buffers of superchunk s-2 must be fully consumed
                    sync.wait_ge(bld_sem, (s - 1) * U)   # DVE (idx + builds)
                    sync.wait_ge(pool_sem, s - 1)        # POOL focal (ep, ye)
                    sync.wait_ge(actE_sem, s - 1)        # ACT abs/exp (ep)
                b = s % 2
                dsem = dma_semA if b == 0 else dma_semB
                cs = slice((s % ns) * U, (s % ns + 1) * U)
                bs = slice(b * U, (b + 1) * U)
                sync.dma_start(out=b_ep[:, bs], in_=ep_ext[:, cs]).then_inc(dsem, 16)
                sync.dma_start(out=b_ye[:, bs], in_=ye_ext[:, cs]).then_inc(dsem, 16)
                sync.dma_start(out=b_dlo[:, bs], in_=dlo_ext[:, cs]).then_inc(dsem, 16)
                sync.dma_start(out=b_dhi[:, bs], in_=dhi_ext[:, cs]).then_inc(dsem, 16)
                sync.dma_start(out=b_slo[:, bs], in_=slo_ext[:, cs]).then_inc(dsem, 16)
                sync.dma_start(out=b_shi[:, bs], in_=shi_ext[:, cs]).then_inc(dsem, 16)

        # ---------------- ACT: |x|, exp, ln (+ tail squares) ----------------
        @block.scalar
        def _(scalar):
            scalar.wait_ge(vset_sem, 1)          # neg1 ready (scale operand)
            for s in range(ns * repeat):
                b = s % 2
                bs = slice(b * U, (b + 1) * U)
                scalar.wait_ge(dma_semA if s % 2 == 0 else dma_semB,
                               (s // 2 + 1) * 96)          # superchunk s loaded
                if s >= 2:
                    # b_t[s-2] readers: POOL t1p, DVE probs
                    scalar.wait_ge(aux_sem, s - 1)
                    scalar.wait_ge(dvp_sem, s - 1)
                scalar.drain()
                scalar.activation(vwA[:, :], b_ep[:, bs], Act.Abs)
                scalar.drain()
                scalar.activation(b_t[:, bs], vwA[:, :], Act.Exp,
                                  scale=neg1[:, :]).then_inc(actE_sem, 1)
                scalar.wait_ge(aux_sem, s + 1)             # 1+t ready
                if s >= 2:
                    scalar.wait_ge(pool_sem, s - 1)        # b_l[s-2] reader: focal
                scalar.activation(b_l[:, bs], b_t1p[:, bs],
                                  Act.Ln).then_inc(actL_sem, 1)
            # ---- tail: squares over reduced bins ----
            scalar.wait_ge(fin_sem, 2)
            scalar.activation(nf_trash[:, :], stats[:, 0:NF], Act.Square,
                              bias=neg1[:, :], accum_out=packed[:, 0:1])
            scalar.drain()
            scalar.activation(nf_trash[:, :], stats[:, NF:2 * NF], Act.Square,
                              bias=neg1[:, :], accum_out=packed[:, 1:2])
            scalar.drain()
            scalar.wait_ge(fin_sem, 3)   # nf_w1 = Gin - Gout ready (DVE)
            scalar.activation(nf_trash[:, :], nf_w1[:, :], Act.Square,
                              accum_out=packed[:, 2:3]).then_inc(fin_sem, 1)  # ->4

        # ---------------- DVE: probs + one-hot builds + tail ----------------
        @block.vector
        def _(vector):
            vector.memset(ones[:, :], 1.0)
            vector.memset(neg1[:, :], -1.0)
            vector.drain()
            vector.engine_nop().then_inc(vset_sem, 1)
            vector.wait_ge(set_sem, 1)           # iotas ready
            for s in range(ns * repeat):
                b = s % 2
                bs = slice(b * U, (b + 1) * U)
                # probs = (x>=0 ? 1 : t) / (1+t)
                vector.wait_ge(dma_semA if s % 2 == 0 else dma_semB,
                               (s // 2 + 1) * 96)
                vector.wait_ge(actE_sem, s + 1)
                vector.wait_ge(aux_sem, s + 1)
                if s >= 2:
                    vector.wait_ge(pool_sem, s - 1)   # b_p[s-2] reader: focal
                vector.drain()
                vector.tensor_scalar(vw1[:, :], b_ep[:, bs], 0.0, None, Alu.is_ge)
                vector.drain()
                vector.tensor_tensor(vw2[:, :], b_t[:, bs], vw1[:, :], Alu.mult)
                vector.drain()
                vector.tensor_tensor(vw2[:, :], b_t[:, bs], vw2[:, :], Alu.subtract)
                vector.drain()
                vector.tensor_tensor(vw2[:, :], vw2[:, :], vw1[:, :], Alu.add)
                vector.reciprocal(vw3[:, :], b_t1p[:, bs])
                vector.drain()
                vector.tensor_tensor(b_p[:, bs], vw2[:, :], vw3[:, :],
                                     Alu.mult).then_inc(dvp_sem, 1)
                vector.drain()
                for j in range(U):
                    g = s * U + j
                    if g >= R and g % 4 == 0:
                        vector.wait_ge(pe_sem, g - R + 4)
                    c = b * U + j
                    vector.tensor_scalar(
                        ring(lhsT_d, g, P), iota128[:, :],
                        b_dlo[:, c:c + 1], b_p[:, c:c + 1],
                        Alu.is_equal, Alu.mult)
                    vector.tensor_scalar(
                        ring(hi_d, g, NF), iota782[:, :],
                        b_dhi[:, c:c + 1], None, Alu.is_equal)
                    vector.tensor_scalar(
                        ring(lhsT_s, g, P), iota128[:, :],
                        b_slo[:, c:c + 1], b_p[:, c:c + 1],
                        Alu.is_equal, Alu.mult)
                    vector.tensor_scalar(
                        ring(hi_s, g, NF), iota782[:, :],
                        b_shi[:, c:c + 1], None, Alu.is_equal).then_inc(bld_sem, 1)

            # ---- tail ----
            vector.wait_ge(pe_sem, nchunk * repeat)   # all matmuls done
            vector.drain()
            vector.tensor_copy(stats[:, 0:NF], ps_in[:, 0:NF])
            vector.drain()
            vector.tensor_copy(stats[:, NF:2 * NF], ps_out[:, 0:NF])
            vector.wait_ge(pool_sem, ns * repeat)     # facc final
            vector.drain()
            vector.tensor_reduce(stats[:, 2 * NF:2 * NF + 1], facc[:, :],
                                 axis=mybir.AxisListType.X, op=Alu.add)
            vector.wait_ge(nod_sem, 64)
            # node mse + count + demand
            vector.drain()
            vector.tensor_scalar(nf_w1[:, :], ynode_t[:, :], 0.0, None, Alu.is_ge)
            vector.drain()
            vector.tensor_tensor(nf_w2[:, :], npred_t[:, :], ynode_t[:, :],
                                 Alu.subtract)
            vector.drain()
            vector.tensor_tensor(nf_w2[:, :], nf_w2[:, :], nf_w2[:, :], Alu.mult)
            vector.drain()
            vector.tensor_tensor(nf_w2[:, :], nf_w2[:, :], nf_w1[:, :], Alu.mult)
            vector.drain()
            vector.tensor_reduce(stats[:, 2 * NF + 1:2 * NF + 2], nf_w2[:, :],
                                 axis=mybir.AxisListType.X, op=Alu.add)
            vector.drain()
            vector.tensor_reduce(stats[:, 2 * NF + 2:2 * NF + 3], nf_w1[:, :],
                                 axis=mybir.AxisListType.X, op=Alu.add)
            vector.drain()
            vector.tensor_reduce(stats[:, 2 * NF + 3:2 * NF + 4], dem_t[:, :],
                                 axis=mybir.AxisListType.X, op=Alu.add)
            vector.drain()
            vector.memset(stats[:, 2 * NF + 4:SW], 0.0)
            vector.drain().then_inc(fin_sem, 1)   # -> 1: stats ready for CC

            # after allreduce (fin_sem=2 from gpsimd): Gin-Gout for tour
            vector.wait_ge(fin_sem, 2)
            vector.drain()
            vector.tensor_tensor(nf_w1[:, :], stats[:, 0:NF], stats[:, NF:2 * NF],
                                 Alu.subtract).then_inc(fin_sem, 1)  # -> 3
            # pack remaining partials (focal, mse, cnt, dem) into packed[:,3:7]
            vector.drain()
            vector.tensor_copy(packed[:, 3:7], stats[:, 2 * NF:2 * NF + 4])
            vector.wait_ge(fin_sem, 4)           # ACT squares done
            vector.drain().then_inc(fin_sem, 1)  # -> 5: packed complete

            # ---- final scalar assembly (after ones-matmul, fin_sem=6) ----
            vector.wait_ge(fin_sem, 6)
            vector.drain()
            vector.tensor_copy(r8[:, 0:7], ps_fin[:, 0:7])
            gin0 = stats[0:1, 0:1]
            gout0 = stats[0:1, NF:NF + 1]
            # coverage corrections: (g0-1)^2 each direction, 192 pad cells
            vector.drain()
            vector.tensor_scalar(sc[:, 0:1], gin0, -1.0, None, Alu.add)
            vector.drain()
            vector.tensor_tensor(sc[:, 0:1], sc[:, 0:1], sc[:, 0:1], Alu.mult)
            vector.drain()
            vector.tensor_scalar(sc[:, 1:2], gout0, -1.0, None, Alu.add)
            vector.drain()
            vector.tensor_tensor(sc[:, 1:2], sc[:, 1:2], sc[:, 1:2], Alu.mult)
            # coverage = (r8[0]+r8[1]-sc0-sc1-192) / (2*(N-1))
            vector.drain()
            vector.tensor_tensor(sc[:, 2:3], r8[:, 0:1], r8[:, 1:2], Alu.add)
            vector.drain()
            vector.tensor_tensor(sc[:, 2:3], sc[:, 2:3], sc[:, 0:1], Alu.subtract)
            vector.drain()
            vector.tensor_tensor(sc[:, 2:3], sc[:, 2:3], sc[:, 1:2], Alu.subtract)
            vector.drain()
            vector.tensor_scalar(sc[:, 2:3], sc[:, 2:3], -192.0,
                                 1.0 / (2.0 * (N_NODES - 1)), Alu.add, Alu.mult)
            vector.drain()
            vector.tensor_scalar(sc[:, 3:4], r8[:, 2:3], 1.0 / N_NODES, None,
                                 Alu.mult)               # tour_formation
            vector.drain()
            vector.tensor_tensor(sc[:, 4:5], gin0, gout0, Alu.subtract)
            vector.drain()
            vector.tensor_tensor(sc[:, 4:5], sc[:, 4:5], sc[:, 4:5], Alu.mult)  # depot
            # expected tours: t = (dem_sum/8 - dem[0]) / cap
            vector.drain()
            vector.tensor_scalar(sc[:, 5:6], r8[:, 6:7], 0.125, None, Alu.mult)
            vector.drain()
            vector.tensor_tensor(sc[:, 5:6], sc[:, 5:6], dem_t[0:1, 0:1], Alu.subtract)
            vector.drain()
            vector.reciprocal(sc[:, 6:7], capsb[:, :])
            vector.drain()
            vector.tensor_tensor(sc[:, 5:6], sc[:, 5:6], sc[:, 6:7], Alu.mult)  # t
            vector.drain()
            vector.tensor_copy(i32t[:, :], sc[:, 5:6])   # fi = int(t)
            vector.drain()
            vector.tensor_copy(sc[:, 7:8], i32t[:, :])
            vector.drain()
            vector.tensor_tensor(sc[:, 8:9], sc[:, 7:8], sc[:, 5:6], Alu.is_lt)
            vector.drain()
            vector.tensor_tensor(sc[:, 7:8], sc[:, 7:8], sc[:, 8:9], Alu.add)  # ceil
            vector.drain()
            vector.tensor_tensor(sc[:, 8:9], gout0, sc[:, 7:8], Alu.subtract)
            vector.drain()
            vector.tensor_tensor(sc[:, 8:9], sc[:, 8:9], sc[:, 8:9], Alu.mult)  # cap_tours
            vector.drain()
            vector.tensor_scalar(sc[:, 9:10], r8[:, 3:4], 1.0 / n_edges_real, None,
                                 Alu.mult)               # similarity
            # node_loss = (mse/8) / max(cnt/8, 1)
            vector.drain()
            vector.tensor_scalar(sc[:, 10:11], r8[:, 4:5], 0.125, None, Alu.mult)
            vector.drain()
            vector.tensor_scalar(sc[:, 11:12], r8[:, 5:6], 0.125, None, Alu.mult)
            vector.drain()
            vector.tensor_scalar(sc[:, 11:12], sc[:, 11:12], 1.0, None, Alu.max)
            vector.drain()
            vector.reciprocal(sc[:, 12:13], sc[:, 11:12])
            vector.drain()
            vector.tensor_tensor(sc[:, 10:11], sc[:, 10:11], sc[:, 12:13], Alu.mult)
            # total
            vector.drain()
            vector.tensor_scalar(outsb[:, :], sc[:, 2:3], 5.0, None, Alu.mult)
            vector.drain()
            vector.tensor_scalar(sc[:, 3:4], sc[:, 3:4], 3.0, None, Alu.mult)
            vector.drain()
            vector.tensor_tensor(outsb[:, :], outsb[:, :], sc[:, 3:4], Alu.add)
            vector.drain()
            vector.tensor_scalar(sc[:, 4:5], sc[:, 4:5], 2.0, None, Alu.mult)
            vector.drain()
            vector.tensor_tensor(outsb[:, :], outsb[:, :], sc[:, 4:5], Alu.add)
            vector.drain()
            vector.tensor_scalar(sc[:, 8:9], sc[:, 8:9], 1.5, None, Alu.mult)
            vector.drain()
            vector.tensor_tensor(outsb[:, :], outsb[:, :], sc[:, 8:9], Alu.add)
            vector.drain()
            vector.tensor_scalar(sc[:, 9:10], sc[:, 9:10], 0.3, None, Alu.mult)
            vector.drain()
            vector.tensor_tensor(outsb[:, :], outsb[:, :], sc[:, 9:10], Alu.add)
            vector.drain()
            vector.tensor_scalar(sc[:, 10:11], sc[:, 10:11], 0.1, None, Alu.mult)
            vector.drain()
            vector.tensor_tensor(outsb[:, :], outsb[:, :], sc[:, 10:11],
                                 Alu.add).then_inc(fin_sem, 1)   # -> 7

        # ---------------- PE: binning matmuls + final ones-reduce ----------------
        @block.tensor
        def _(tensor):
            for g in range(nchunk * repeat):
                tensor.wait_ge(bld_sem, g + 1)
                st = (g == 0)
                sp = (g == nchunk * repeat - 1)
                ld = ring(lhsT_d, g, P)
                ls = ring(lhsT_s, g, P)
                hd = ring(hi_d, g, NF)
                hs = ring(hi_s, g, NF)
                tensor.matmul(ps_in[:, 0:512], ld, hd[:, 0:512],
                              start=st, stop=sp, skip_group_check=True)
                tensor.matmul(ps_in[:, 512:NF], ld, hd[:, 512:NF],
                              start=st, stop=sp, skip_group_check=True)
                tensor.matmul(ps_out[:, 0:512], ls, hs[:, 0:512],
                              start=st, stop=sp, skip_group_check=True)
                tensor.matmul(ps_out[:, 512:NF], ls, hs[:, 512:NF],
                              start=st, stop=sp,
                              skip_group_check=True).then_inc(pe_sem, 1)
            # final partition reduce of packed stats via ones-matmul
            tensor.wait_ge(fin_sem, 5)
            tensor.matmul(ps_fin[:, 0:7], ones[:, :], packed[:, 0:7],
                          start=True, stop=True,
                          skip_group_check=True).then_inc(fin_sem, 1)  # -> 6

        # ---------------- POOL: iotas, 1+t, focal, collective ----------------
        @block.gpsimd
        def _(gpsimd):
            gpsimd.iota(iota128[:, :], pattern=[[1, P]], base=0, channel_multiplier=0,
                        allow_small_or_imprecise_dtypes=True)
            gpsimd.iota(iota782[:, :], pattern=[[1, NF]], base=0, channel_multiplier=0,
                        allow_small_or_imprecise_dtypes=True)
            gpsimd.memset(facc[:, :], 0.0).then_inc(set_sem, 1)
            for s in range(ns * repeat):
                b = s % 2
                bs = slice(b * U, (b + 1) * U)
                # 1 + t (t from ACT)
                gpsimd.wait_ge(actE_sem, s + 1)
                if s >= 2:
                    gpsimd.wait_ge(actL_sem, s - 1)  # b_t1p[s-2] reader: ACT ln
                    gpsimd.wait_ge(dvp_sem, s - 1)   # b_t1p[s-2] reader: DVE recip
                gpsimd.tensor_scalar(b_t1p[:, bs], b_t[:, bs], 1.0, None,
                                     Alu.add).then_inc(aux_sem, 1)
                # focal: facc += (0.75-0.5y)*u^2*(relu(x) - x*y + ln1p)
                gpsimd.wait_ge(dma_semA if s % 2 == 0 else dma_semB,
                               (s // 2 + 1) * 96)
                gpsimd.wait_ge(actL_sem, s + 1)
                gpsimd.wait_ge(dvp_sem, s + 1)
                gpsimd.drain()
                gpsimd.tensor_tensor(w1[:, :], b_p[:, bs], b_ye[:, bs], Alu.mult)
                gpsimd.tensor_tensor(w2[:, :], b_p[:, bs], b_ye[:, bs], Alu.add)
                gpsimd.drain()
                gpsimd.tensor_scalar(w1[:, :], w1[:, :], -2.0, None, Alu.mult)
                gpsimd.drain()
                gpsimd.tensor_tensor(w2[:, :], w2[:, :], w1[:, :], Alu.add)
                gpsimd.drain()
                gpsimd.tensor_tensor(w2[:, :], w2[:, :], w2[:, :], Alu.mult)
                gpsimd.tensor_scalar(w3[:, :], b_ye[:, bs], -0.5, 0.75,
                                     Alu.mult, Alu.add)
                gpsimd.drain()
                gpsimd.tensor_tensor(w2[:, :], w2[:, :], w3[:, :], Alu.mult)
                gpsimd.tensor_scalar(w1[:, :], b_ep[:, bs], 0.0, None, Alu.max)
                gpsimd.drain()
                gpsimd.tensor_tensor(w3[:, :], b_ep[:, bs], b_ye[:, bs], Alu.mult)
                gpsimd.drain()
                gpsimd.tensor_tensor(w1[:, :], w1[:, :], w3[:, :], Alu.subtract)
                gpsimd.drain()
                gpsimd.tensor_tensor(w1[:, :], w1[:, :], b_l[:, bs], Alu.add)
                gpsimd.drain()
                gpsimd.tensor_tensor(w2[:, :], w2[:, :], w1[:, :], Alu.mult)
                gpsimd.drain()
                gpsimd.tensor_tensor(facc[:, :], facc[:, :], w2[:, :],
                                     Alu.add).then_inc(pool_sem, 1)
            # collective
            gpsimd.wait_ge(fin_sem, 1)
            gpsimd.dma_start(out=cc_in[:, :], in_=stats[:, :]).then_inc(odma_sem, 16)
            gpsimd.wait_ge(odma_sem, 16)
            gpsimd.collective_compute(
                "AllReduce", Alu.add,
                replica_groups=[list(range(NCORES))],
                ins=[cc_in[:, :]], outs=[cc_out[:, :]],
            ).then_inc(cc_sem, 1)
            gpsimd.wait_ge(cc_sem, 1)
            gpsimd.dma_start(out=stats[:, :], in_=cc_out[:, :]).then_inc(odma_sem, 16)
            gpsimd.wait_ge(odma_sem, 32)
            gpsimd.engine_nop().then_inc(fin_sem, 1)   # -> 2: reduced stats in SBUF
            gpsimd.wait_ge(fin_sem, 7)
            gpsimd.dma_start(out=out_ext[:, :], in_=outsb[:, :]).then_inc(odma_sem, 16)
            gpsimd.wait_ge(odma_sem, 48)

    return nc


def _prep_shards(edge_predictions, node_predictions, x, capacity, y_edges,
                 y_nodes, edge_index, ncols=NCOLS_FULL):
    ep = np.asarray(edge_predictions, np.float32).ravel()
    ye = np.asarray(y_edges, np.float32).ravel()
    ei = np.asarray(edge_index)
    src = ei[0].astype(np.int64)
    dst = ei[1].astype(np.int64)
    npred = np.asarray(node_predictions, np.float32).ravel()
    ynode = np.asarray(y_nodes, np.float32).ravel()
    dem = np.asarray(x, np.float32)[:, 2].ravel()

    # node arrays padded to 128*782
    npad = P * NF - N_NODES
    np_t = np.concatenate([npred, np.zeros(npad, np.float32)]).reshape(P, NF)
    yn_t = np.concatenate([ynode, np.full(npad, -1.0, np.float32)]).reshape(P, NF)
    dem_t = np.concatenate([dem, np.zeros(npad, np.float32)]).reshape(P, NF)
    cap = np.float32(np.asarray(capacity, np.float32).mean()).reshape(1, 1)

    n_edges = ep.shape[0]
    epc = n_edges // NCORES
    maps = []
    for c in range(NCORES):
        sl = slice(c * epc, (c + 1) * epc)

        def padded(a, fill, dtype):
            out = np.full(ncols * P, fill, dtype)
            out[:epc] = a[sl].astype(dtype)
            return out.reshape(P, ncols)

        maps.append({
            "ep": padded(ep, PAD_LOGIT, np.float32),
            "ye": padded(ye, 0.0, np.float32),
            "dlo": padded(dst & 127, 0, np.float32),
            "dhi": padded(dst >> 7, 0, np.float32),
            "slo": padded(src & 127, 0, np.float32),
            "shi": padded(src >> 7, 0, np.float32),
            "npred": np_t,
            "ynode": yn_t,
            "dem": dem_t,
            "cap": cap,
        })
    return maps


_NC_CACHE = {}


def kernel(edge_predictions, node_predictions, x, capacity, y_edges, y_nodes,
           edge_index, num_nodes):
    maps = _prep_shards(edge_predictions, node_predictions, x, capacity,
                        y_edges, y_nodes, edge_index)
    if "nc" not in _NC_CACHE:
        _NC_CACHE["nc"] = build_nc()
    nc = _NC_CACHE["nc"]
    res = run_bass_kernel_spmd(nc, maps, list(range(NCORES)))
    val = np.float32(res.results[0]["out"].reshape(-1)[0])
    return np.asarray(val, dtype=np.float32)

